# revision 4
# baseline (speedup 1.0000x reference)
"""Bilinear interpolation kernel for Trainium2 (8 NeuronCores, SPMD).

Strategy (data-parallel over query points, image replicated per core):
  * Host shards coords [2, N] into 8 equal slices of N/8 points.
  * Device phase 1: build an interleaved "pairs" table in DRAM:
      pairs[y*W + x] = (img[y, x], img[y+1, x])   for y in [0, H-2]
    so the 4 bilinear corners for (y0, x0) are the 4 consecutive floats
    at pairs.flat[2*(y0*W + x0) : +4] -> one gather descriptor per point.
  * Device phase 2: stream coord chunks, compute floor/clamp/weights/flat
    index on VectorE, gather 16B per point via indirect DMA (128 points
    per instruction, one descriptor per SBUF partition), blend, mask,
    stream out values (f32) + valid (u8).
"""
import sys

sys.path.insert(0, "/opt/trn_rl_repo")

from contextlib import ExitStack

import numpy as np

import concourse.bass as bass
import concourse.mybir as mybir
from concourse.bass_utils import run_bass_kernel_spmd

H = W = 4096
N = 16777216
NCORES = 8
NPC = N // NCORES            # points per core (2_097_152)
PP = 128                     # partitions
K = 512                      # points per partition per superchunk
CH = PP * K                  # superchunk points (65536)
NS = NPC // CH               # superchunks (32)
PROWS = (H - 1) * W          # pairs table rows
MAGIC = 8388608.0            # 2^23 fp32 round-to-int magic

f32 = mybir.dt.float32
i32 = mybir.dt.int32
u8 = mybir.dt.uint8


def build_nc(npc=NPC):
    ns = npc // CH
    nc = bass.Bass()
    x_d = nc.declare_dram_parameter("x", [H, W], f32, isOutput=False)
    xq_d = nc.declare_dram_parameter("xq", [npc], f32, isOutput=False)
    yq_d = nc.declare_dram_parameter("yq", [npc], f32, isOutput=False)
    val_d = nc.declare_dram_parameter("values", [npc], f32, isOutput=True)
    vld_d = nc.declare_dram_parameter("valid", [npc], u8, isOutput=True)
    pairs_d = nc.dram_tensor("pairs", [PROWS, 2], f32)

    es = ExitStack()
    with (
        nc.semaphore("p_in") as p_in,      # build: img tiles loaded
        nc.semaphore("p_cp") as p_cp,      # build: interleave done
        nc.semaphore("p_out") as p_out,    # build: pairs tile stored
        nc.semaphore("cin") as cin,        # coords chunk loaded (32/chunk)
        nc.semaphore("wdone") as wdone,    # phase-A compute done (1/chunk)
        nc.semaphore("gdone") as gdone,    # gathers done (16*K/chunk)
        nc.semaphore("bdone") as bdone,    # blend done (1/chunk)
        nc.semaphore("odone") as odone,    # outputs stored (32/chunk)
        nc.Block() as block,
        es,
    ):
        def sb(name, shape, dt):
            return es.enter_context(nc.sbuf_tensor(name, shape, dt))

        # build phase tiles (2 bufs each)
        ia = [sb(f"ia{b}", [PP, W], f32) for b in range(2)]
        ib = [sb(f"ib{b}", [PP, W], f32) for b in range(2)]
        pt = [sb(f"pt{b}", [PP, 2 * W], f32) for b in range(2)]
        # main loop tiles
        txq = [sb(f"txq{b}", [PP, K], f32) for b in range(2)]
        tyq = [sb(f"tyq{b}", [PP, K], f32) for b in range(2)]
        tt = sb("tt", [PP, K], f32)
        tgt = sb("tgt", [PP, K], f32)
        tfx = sb("tfx", [PP, K], f32)
        tfy = sb("tfy", [PP, K], f32)
        txc = sb("txc", [PP, K], f32)
        tyc = sb("tyc", [PP, K], f32)
        tvf = [sb(f"tvf{b}", [PP, K], f32) for b in range(2)]
        tif = sb("tif", [PP, K], f32)
        tidx = [sb(f"tidx{b}", [PP, K], i32) for b in range(2)]
        ax0 = [sb(f"ax0{b}", [PP, K], f32) for b in range(2)]
        ax1 = [sb(f"ax1{b}", [PP, K], f32) for b in range(2)]
        ay0 = [sb(f"ay0{b}", [PP, K], f32) for b in range(2)]
        ay1 = [sb(f"ay1{b}", [PP, K], f32) for b in range(2)]
        gt_ = [sb(f"g{b}", [PP, 4 * K], f32) for b in range(2)]
        acc = [sb(f"acc{b}", [PP, K], f32) for b in range(2)]
        tw = sb("tw", [PP, K], f32)
        ts_ = sb("ts", [PP, K], f32)
        tvu = [sb(f"tvu{b}", [PP, K], u8) for b in range(2)]

        # pairs build geometry: chunks of 128 rows over H-1 = 4095 rows
        NB = (H - 1 + PP - 1) // PP          # 32 chunks (last has 127 rows)

        def rows(s):
            r0 = s * PP
            return r0, min(PP, (H - 1) - r0)

        xq_t = xq_d[:].rearrange("(s p k) -> s p k", p=PP, k=K)
        yq_t = yq_d[:].rearrange("(s p k) -> s p k", p=PP, k=K)
        val_t = val_d[:].rearrange("(s p k) -> s p k", p=PP, k=K)
        vld_t = vld_d[:].rearrange("(s p k) -> s p k", p=PP, k=K)

        @block.sync
        def _(sync):
            # ---------- phase 1: pairs table build ----------
            for s in range(NB):
                r0, nr = rows(s)
                b = s % 2
                if s >= 2:
                    sync.wait_ge(p_cp, s - 1)  # interleave of s-2 done -> ia/ib free
                sync.dma_start(out=ia[b][:nr, :], in_=x_d[r0:r0 + nr, :]).then_inc(p_in, 16)
                sync.dma_start(out=ib[b][:nr, :], in_=x_d[r0 + 1:r0 + 1 + nr, :]).then_inc(p_in, 16)
                sync.wait_ge(p_cp, s + 1)
                sync.dma_start(
                    out=pairs_d[r0 * W:(r0 + nr) * W, :]
                    .rearrange("r two -> (r two)")
                    .rearrange("(p q) -> p q", p=nr),
                    in_=pt[b][:nr, :],
                ).then_inc(p_out, 16)
            sync.wait_ge(p_out, 16 * NB)

            # ---------- phase 2: main loop ----------
            for s in range(ns):
                if s >= 2:
                    sync.wait_ge(wdone, s - 1)   # coord bufs free
                sync.dma_start(out=txq[s % 2][:], in_=xq_t[s]).then_inc(cin, 16)
                sync.dma_start(out=tyq[s % 2][:], in_=yq_t[s]).then_inc(cin, 16)
                if s >= 2:
                    sync.wait_ge(bdone, s - 1)
                    sync.dma_start(out=val_t[s - 2], in_=acc[s % 2][:]).then_inc(odone, 16)
                    sync.dma_start(out=vld_t[s - 2], in_=tvu[s % 2][:]).then_inc(odone, 16)
            for s in range(max(0, ns - 2), ns):
                sync.wait_ge(bdone, s + 1)
                sync.dma_start(out=val_t[s], in_=acc[s % 2][:]).then_inc(odone, 16)
                sync.dma_start(out=vld_t[s], in_=tvu[s % 2][:]).then_inc(odone, 16)
            sync.wait_ge(odone, 32 * ns)

        @block.vector
        def _(vector):
            A = mybir.AluOpType
            # ---------- phase 1: interleave pairs ----------
            for s in range(NB):
                r0, nr = rows(s)
                b = s % 2
                vector.wait_ge(p_in, 32 * (s + 1))
                if s >= 2:
                    vector.wait_ge(p_out, 16 * (s - 1))  # pt buf free
                vector.tensor_copy(pt[b][:nr, 0:2 * W:2], ia[b][:nr, :])
                vector.tensor_copy(pt[b][:nr, 1:2 * W:2], ib[b][:nr, :]).then_inc(p_cp, 1)

            # ---------- phase 2 ----------
            def phase_a(s):
                b = s % 2
                vector.wait_ge(cin, 32 * (s + 1))
                xq, yq = txq[b][:], tyq[b][:]
                # floor(xq) -> tfx
                vector.tensor_scalar_add(tt[:], xq, MAGIC)
                vector.tensor_scalar_sub(tt[:], tt[:], MAGIC)
                vector.tensor_tensor(out=tgt[:], in0=tt[:], in1=xq, op=A.is_gt)
                vector.tensor_tensor(out=tfx[:], in0=tt[:], in1=tgt[:], op=A.subtract)
                # floor(yq) -> tfy
                vector.tensor_scalar_add(tt[:], yq, MAGIC)
                vector.tensor_scalar_sub(tt[:], tt[:], MAGIC)
                vector.tensor_tensor(out=tgt[:], in0=tt[:], in1=yq, op=A.is_gt)
                vector.tensor_tensor(out=tfy[:], in0=tt[:], in1=tgt[:], op=A.subtract)
                # clamps + validity
                vector.tensor_scalar(txc[:], tfx[:], 0.0, float(W - 2), A.max, A.min)
                vector.tensor_scalar(tyc[:], tfy[:], 0.0, float(H - 2), A.max, A.min)
                vector.tensor_tensor(out=tt[:], in0=txc[:], in1=tfx[:], op=A.is_equal)
                vector.tensor_tensor(out=tgt[:], in0=tyc[:], in1=tfy[:], op=A.is_equal)
                vector.tensor_tensor(out=tvf[b][:], in0=tt[:], in1=tgt[:], op=A.mult)
                # flat index = yc*W + xc (exact in fp32), cast int32
                vector.tensor_scalar_mul(tif[:], tyc[:], float(W))
                vector.tensor_tensor(out=tif[:], in0=tif[:], in1=txc[:], op=A.add)
                vector.tensor_copy(tidx[b][:], tif[:])
                # weights
                vector.tensor_tensor(out=ax1[b][:], in0=xq, in1=tfx[:], op=A.subtract)
                vector.tensor_scalar(ax0[b][:], ax1[b][:], -1.0, 1.0, A.mult, A.add)
                vector.tensor_tensor(out=ay1[b][:], in0=yq, in1=tfy[:], op=A.subtract)
                vector.tensor_scalar(ay0[b][:], ay1[b][:], -1.0, 1.0, A.mult, A.add) \
                    .then_inc(wdone, 1)

            def phase_b(s):
                b = s % 2
                vector.wait_ge(gdone, 16 * K * (s + 1))
                if s >= 2:
                    vector.wait_ge(odone, 32 * (s - 1))  # acc/tvu bufs free
                G = gt_[b][:]
                v00 = G[:, 0:4 * K:4]
                v01 = G[:, 1:4 * K:4]
                v10 = G[:, 2:4 * K:4]
                v11 = G[:, 3:4 * K:4]
                # values = ax0*ay0*v00 + ax1*ay0*v10 + ax0*ay1*v01 + ax1*ay1*v11
                vector.tensor_tensor(out=tw[:], in0=ax0[b][:], in1=ay0[b][:], op=A.mult)
                vector.tensor_tensor(out=acc[b][:], in0=tw[:], in1=v00, op=A.mult)
                vector.tensor_tensor(out=tw[:], in0=ax1[b][:], in1=ay0[b][:], op=A.mult)
                vector.tensor_tensor(out=ts_[:], in0=tw[:], in1=v10, op=A.mult)
                vector.tensor_tensor(out=acc[b][:], in0=acc[b][:], in1=ts_[:], op=A.add)
                vector.tensor_tensor(out=tw[:], in0=ax0[b][:], in1=ay1[b][:], op=A.mult)
                vector.tensor_tensor(out=ts_[:], in0=tw[:], in1=v01, op=A.mult)
                vector.tensor_tensor(out=acc[b][:], in0=acc[b][:], in1=ts_[:], op=A.add)
                vector.tensor_tensor(out=tw[:], in0=ax1[b][:], in1=ay1[b][:], op=A.mult)
                vector.tensor_tensor(out=ts_[:], in0=tw[:], in1=v11, op=A.mult)
                vector.tensor_tensor(out=acc[b][:], in0=acc[b][:], in1=ts_[:], op=A.add)
                vector.tensor_tensor(out=acc[b][:], in0=acc[b][:], in1=tvf[b][:], op=A.mult)
                vector.tensor_copy(tvu[b][:], tvf[b][:]).then_inc(bdone, 1)

            phase_a(0)
            for s in range(ns):
                if s + 1 < ns:
                    phase_a(s + 1)
                phase_b(s)

        @block.gpsimd
        def _(gpsimd):
            for s in range(ns):
                gpsimd.wait_ge(wdone, s + 1)
                if s >= 2:
                    gpsimd.wait_ge(bdone, s - 1)   # G buf free
                b = s % 2
                for j in range(K):
                    gpsimd.indirect_dma_start(
                        out=gt_[b][:, 4 * j:4 * j + 4],
                        out_offset=None,
                        in_=pairs_d[:],
                        in_offset=bass.IndirectOffsetOnAxis(ap=tidx[b][:, j:j + 1], axis=0),
                    ).then_inc(gdone, 16)

    return nc


_nc_cache = None


def kernel(x: np.ndarray, coords: np.ndarray):
    global _nc_cache
    if _nc_cache is None:
        _nc_cache = build_nc()
    nc = _nc_cache

    x = np.ascontiguousarray(np.asarray(x), dtype=np.float32)
    coords = np.asarray(coords, dtype=np.float32)
    in_maps = []
    for c in range(NCORES):
        sl = slice(c * NPC, (c + 1) * NPC)
        in_maps.append({
            "x": x,
            "xq": np.ascontiguousarray(coords[0, sl]),
            "yq": np.ascontiguousarray(coords[1, sl]),
        })
    res = run_bass_kernel_spmd(nc, in_maps, list(range(NCORES)))
    values = np.concatenate([res.results[c]["values"] for c in range(NCORES)])
    valid = np.concatenate([res.results[c]["valid"] for c in range(NCORES)]).astype(bool)
    return values, valid


# revision 5
# speedup vs baseline: 3.8999x; 3.8999x over previous
"""Bilinear interpolation kernel for Trainium2 (8 NeuronCores, SPMD).

Strategy (data-parallel over query points, image replicated per core):
  * Host shards coords [2, N] into 8 equal slices of N/8 points.
  * Device phase 1: build an interleaved "pairs" table in DRAM:
      pairs[y*W + x] = (img[y, x], img[y+1, x])   for y in [0, H-2]
    so the 4 bilinear corners for (y0, x0) are the 4 consecutive floats
    at pairs.flat[2*(y0*W + x0) : +4] -> one gather descriptor per point.
  * Device phase 2: stream coord chunks, compute floor/clamp/weights/flat
    index on VectorE, gather 16B per point via indirect DMA (128 points
    per instruction, one descriptor per SBUF partition), blend, mask,
    stream out values (f32) + valid (u8).
"""
import sys

sys.path.insert(0, "/opt/trn_rl_repo")

from contextlib import ExitStack

import numpy as np

import concourse.bass as bass
import concourse.mybir as mybir
from concourse.bass_utils import run_bass_kernel_spmd

H = W = 4096
N = 16777216
NCORES = 8
NPC = N // NCORES            # points per core (2_097_152)
PP = 128                     # partitions
K = 512                      # points per partition per superchunk
CH = PP * K                  # superchunk points (65536)
NS = NPC // CH               # superchunks (32)
PROWS = (H - 1) * W          # pairs table rows
MAGIC = 8388608.0            # 2^23 fp32 round-to-int magic

f32 = mybir.dt.float32
i32 = mybir.dt.int32
u8 = mybir.dt.uint8


def build_nc(npc=NPC, reps=1):
    ns = npc // CH
    nc = bass.Bass()
    x_d = nc.declare_dram_parameter("x", [H, W], f32, isOutput=False)
    xq_d = nc.declare_dram_parameter("xq", [npc], f32, isOutput=False)
    yq_d = nc.declare_dram_parameter("yq", [npc], f32, isOutput=False)
    val_d = nc.declare_dram_parameter("values", [npc], f32, isOutput=True)
    vld_d = nc.declare_dram_parameter("valid", [npc], u8, isOutput=True)
    pairs_d = nc.dram_tensor("pairs", [PROWS, 2], f32)

    es = ExitStack()
    with (
        nc.semaphore("p_in") as p_in,      # build: img tiles loaded
        nc.semaphore("p_cp") as p_cp,      # build: interleave done
        nc.semaphore("p_out") as p_out,    # build: pairs tile stored
        nc.semaphore("cin") as cin,        # coords chunk loaded (32/chunk)
        nc.semaphore("wdone") as wdone,    # phase-A compute done (1/chunk)
        nc.semaphore("gdone") as gdone,    # gathers done (16*K/chunk)
        nc.semaphore("bdone") as bdone,    # blend done (1/chunk)
        nc.semaphore("odone") as odone,    # outputs stored (32/chunk)
        nc.Block() as block,
        es,
    ):
        def sb(name, shape, dt):
            return es.enter_context(nc.sbuf_tensor(name, shape, dt))

        # build phase tiles (2 bufs each)
        ia = [sb(f"ia{b}", [PP, W], f32) for b in range(2)]
        ib = [sb(f"ib{b}", [PP, W], f32) for b in range(2)]
        pt = [sb(f"pt{b}", [PP, 2 * W], f32) for b in range(2)]
        # main loop tiles
        txq = [sb(f"txq{b}", [PP, K], f32) for b in range(2)]
        tyq = [sb(f"tyq{b}", [PP, K], f32) for b in range(2)]
        tt = sb("tt", [PP, K], f32)
        tgt = sb("tgt", [PP, K], f32)
        tfx = sb("tfx", [PP, K], f32)
        tfy = sb("tfy", [PP, K], f32)
        txc = sb("txc", [PP, K], f32)
        tyc = sb("tyc", [PP, K], f32)
        tvf = [sb(f"tvf{b}", [PP, K], f32) for b in range(2)]
        tif = sb("tif", [PP, K], f32)
        tidx = [sb(f"tidx{b}", [PP, K], i32) for b in range(2)]
        ax0 = [sb(f"ax0{b}", [PP, K], f32) for b in range(2)]
        ax1 = [sb(f"ax1{b}", [PP, K], f32) for b in range(2)]
        ay0 = [sb(f"ay0{b}", [PP, K], f32) for b in range(2)]
        ay1 = [sb(f"ay1{b}", [PP, K], f32) for b in range(2)]
        gt_ = [sb(f"g{b}", [PP, 4 * K], f32) for b in range(2)]
        acc = [sb(f"acc{b}", [PP, K], f32) for b in range(2)]
        tw = sb("tw", [PP, K], f32)
        ts_ = sb("ts", [PP, K], f32)
        tvu = [sb(f"tvu{b}", [PP, K], u8) for b in range(2)]

        # pairs build geometry: chunks of 128 rows over H-1 = 4095 rows
        NB = (H - 1 + PP - 1) // PP          # 32 chunks (last has 127 rows)

        def rows(s):
            r0 = s * PP
            return r0, min(PP, (H - 1) - r0)

        xq_t = xq_d[:].rearrange("(s p k) -> s p k", p=PP, k=K)
        yq_t = yq_d[:].rearrange("(s p k) -> s p k", p=PP, k=K)
        val_t = val_d[:].rearrange("(s p k) -> s p k", p=PP, k=K)
        vld_t = vld_d[:].rearrange("(s p k) -> s p k", p=PP, k=K)

        @block.sync
        def _(sync):
            # ---------- phase 1: pairs table build ----------
            for s in range(NB):
                r0, nr = rows(s)
                b = s % 2
                if s >= 2:
                    sync.wait_ge(p_cp, s - 1)  # interleave of s-2 done -> ia/ib free
                sync.dma_start(out=ia[b][:nr, :], in_=x_d[r0:r0 + nr, :]).then_inc(p_in, 16)
                sync.dma_start(out=ib[b][:nr, :], in_=x_d[r0 + 1:r0 + 1 + nr, :]).then_inc(p_in, 16)
                sync.wait_ge(p_cp, s + 1)
                sync.dma_start(
                    out=pairs_d[r0 * W:(r0 + nr) * W, :]
                    .rearrange("r two -> (r two)")
                    .rearrange("(p q) -> p q", p=nr),
                    in_=pt[b][:nr, :],
                ).then_inc(p_out, 16)
            sync.wait_ge(p_out, 16 * NB)

            # ---------- phase 2: main loop (reps x, global chunk index g) ----------
            NG = reps * ns
            for g in range(NG):
                s = g % ns
                if g >= 2:
                    sync.wait_ge(wdone, g - 1)   # coord bufs free
                sync.dma_start(out=txq[g % 2][:], in_=xq_t[s]).then_inc(cin, 16)
                sync.dma_start(out=tyq[g % 2][:], in_=yq_t[s]).then_inc(cin, 16)
                if g >= 2:
                    sync.wait_ge(bdone, g - 1)
                    sync.dma_start(out=val_t[(g - 2) % ns], in_=acc[g % 2][:]).then_inc(odone, 16)
                    sync.dma_start(out=vld_t[(g - 2) % ns], in_=tvu[g % 2][:]).then_inc(odone, 16)
            for g in range(max(0, NG - 2), NG):
                sync.wait_ge(bdone, g + 1)
                sync.dma_start(out=val_t[g % ns], in_=acc[g % 2][:]).then_inc(odone, 16)
                sync.dma_start(out=vld_t[g % ns], in_=tvu[g % 2][:]).then_inc(odone, 16)
            sync.wait_ge(odone, 32 * NG)

        @block.vector
        def _(vector):
            A = mybir.AluOpType
            # ---------- phase 1: interleave pairs ----------
            for s in range(NB):
                r0, nr = rows(s)
                b = s % 2
                vector.wait_ge(p_in, 32 * (s + 1))
                if s >= 2:
                    vector.wait_ge(p_out, 16 * (s - 1))  # pt buf free
                vector.tensor_copy(pt[b][:nr, 0:2 * W:2], ia[b][:nr, :])
                vector.tensor_copy(pt[b][:nr, 1:2 * W:2], ib[b][:nr, :]).then_inc(p_cp, 1)

            # ---------- phase 2 ----------
            def phase_a(g):
                s = g % ns
                b = g % 2
                vector.wait_ge(cin, 32 * (g + 1))
                xq, yq = txq[b][:], tyq[b][:]
                # floor(xq) -> tfx
                vector.tensor_scalar_add(tt[:], xq, MAGIC)
                vector.tensor_scalar_sub(tt[:], tt[:], MAGIC)
                vector.tensor_tensor(out=tgt[:], in0=tt[:], in1=xq, op=A.is_gt)
                vector.tensor_tensor(out=tfx[:], in0=tt[:], in1=tgt[:], op=A.subtract)
                # floor(yq) -> tfy
                vector.tensor_scalar_add(tt[:], yq, MAGIC)
                vector.tensor_scalar_sub(tt[:], tt[:], MAGIC)
                vector.tensor_tensor(out=tgt[:], in0=tt[:], in1=yq, op=A.is_gt)
                vector.tensor_tensor(out=tfy[:], in0=tt[:], in1=tgt[:], op=A.subtract)
                # clamps + validity
                vector.tensor_scalar(txc[:], tfx[:], 0.0, float(W - 2), A.max, A.min)
                vector.tensor_scalar(tyc[:], tfy[:], 0.0, float(H - 2), A.max, A.min)
                vector.tensor_tensor(out=tt[:], in0=txc[:], in1=tfx[:], op=A.is_equal)
                vector.tensor_tensor(out=tgt[:], in0=tyc[:], in1=tfy[:], op=A.is_equal)
                vector.tensor_tensor(out=tvf[b][:], in0=tt[:], in1=tgt[:], op=A.mult)
                # flat index = yc*W + xc (exact in fp32), cast int32
                vector.tensor_scalar_mul(tif[:], tyc[:], float(W))
                vector.tensor_tensor(out=tif[:], in0=tif[:], in1=txc[:], op=A.add)
                vector.tensor_copy(tidx[b][:], tif[:])
                # weights
                vector.tensor_tensor(out=ax1[b][:], in0=xq, in1=tfx[:], op=A.subtract)
                vector.tensor_scalar(ax0[b][:], ax1[b][:], -1.0, 1.0, A.mult, A.add)
                vector.tensor_tensor(out=ay1[b][:], in0=yq, in1=tfy[:], op=A.subtract)
                vector.tensor_scalar(ay0[b][:], ay1[b][:], -1.0, 1.0, A.mult, A.add) \
                    .then_inc(wdone, 1)

            def phase_b(g):
                b = g % 2
                vector.wait_ge(gdone, 16 * K * (g + 1))
                if g >= 2:
                    vector.wait_ge(odone, 32 * (g - 1))  # acc/tvu bufs free
                G = gt_[b][:]
                v00 = G[:, 0:4 * K:4]
                v01 = G[:, 1:4 * K:4]
                v10 = G[:, 2:4 * K:4]
                v11 = G[:, 3:4 * K:4]
                # values = ax0*ay0*v00 + ax1*ay0*v10 + ax0*ay1*v01 + ax1*ay1*v11
                vector.tensor_tensor(out=tw[:], in0=ax0[b][:], in1=ay0[b][:], op=A.mult)
                vector.tensor_tensor(out=acc[b][:], in0=tw[:], in1=v00, op=A.mult)
                vector.tensor_tensor(out=tw[:], in0=ax1[b][:], in1=ay0[b][:], op=A.mult)
                vector.tensor_tensor(out=ts_[:], in0=tw[:], in1=v10, op=A.mult)
                vector.tensor_tensor(out=acc[b][:], in0=acc[b][:], in1=ts_[:], op=A.add)
                vector.tensor_tensor(out=tw[:], in0=ax0[b][:], in1=ay1[b][:], op=A.mult)
                vector.tensor_tensor(out=ts_[:], in0=tw[:], in1=v01, op=A.mult)
                vector.tensor_tensor(out=acc[b][:], in0=acc[b][:], in1=ts_[:], op=A.add)
                vector.tensor_tensor(out=tw[:], in0=ax1[b][:], in1=ay1[b][:], op=A.mult)
                vector.tensor_tensor(out=ts_[:], in0=tw[:], in1=v11, op=A.mult)
                vector.tensor_tensor(out=acc[b][:], in0=acc[b][:], in1=ts_[:], op=A.add)
                vector.tensor_tensor(out=acc[b][:], in0=acc[b][:], in1=tvf[b][:], op=A.mult)
                vector.tensor_copy(tvu[b][:], tvf[b][:]).then_inc(bdone, 1)

            NG = reps * ns
            phase_a(0)
            for g in range(NG):
                if g + 1 < NG:
                    phase_a(g + 1)
                phase_b(g)

        @block.gpsimd
        def _(gpsimd):
            for g in range(reps * ns):
                gpsimd.wait_ge(wdone, g + 1)
                if g >= 2:
                    gpsimd.wait_ge(bdone, g - 1)   # G buf free
                b = g % 2
                for j in range(K):
                    gpsimd.indirect_dma_start(
                        out=gt_[b][:, 4 * j:4 * j + 4],
                        out_offset=None,
                        in_=pairs_d[:],
                        in_offset=bass.IndirectOffsetOnAxis(ap=tidx[b][:, j:j + 1], axis=0),
                    ).then_inc(gdone, 16)

    return nc


_nc_cache = None


def kernel(x: np.ndarray, coords: np.ndarray):
    global _nc_cache
    if _nc_cache is None:
        _nc_cache = build_nc()
    nc = _nc_cache

    x = np.ascontiguousarray(np.asarray(x), dtype=np.float32)
    coords = np.asarray(coords, dtype=np.float32)
    in_maps = []
    for c in range(NCORES):
        sl = slice(c * NPC, (c + 1) * NPC)
        in_maps.append({
            "x": x,
            "xq": np.ascontiguousarray(coords[0, sl]),
            "yq": np.ascontiguousarray(coords[1, sl]),
        })
    res = run_bass_kernel_spmd(nc, in_maps, list(range(NCORES)))
    values = np.concatenate([res.results[c]["values"] for c in range(NCORES)])
    valid = np.concatenate([res.results[c]["valid"] for c in range(NCORES)]).astype(bool)
    return values, valid
